# revision 21
# baseline (speedup 1.0000x reference)
"""NonLocalBlock Trainium2 kernel (v3).

8-core split: data-parallel over batch B=4 (2 cores per batch element),
each core pair splits the [N,N] score matrix by rows n (core r owns
n in [2048r, 2048r+2048)). Scores are computed transposed (ST[m,n]) so
both output matmuls contract over m with m on partitions. The output
1x1 convs are folded in before the attention matmuls via
Z = (w_o @ X3v^T)^T, so no on-device transposes are needed anywhere.
Softmaxes use a constant shift (no per-row max): exp(s-64) is safe for
randn-scale inputs, and constant shifts cancel exactly in softmax.

Pipeline structure (all engines near-saturated):
- x / projection weights arrive fp16 (host-converted); all phase-P
  matmuls are fp16 at full PE rate with no staging copies.
- Phase P interleaves the X2 and X1/X3 projection streams so the PE
  stays continuously busy (ramps to 2.4 GHz); PSUM->SBUF copies are
  spread across Scalar/Vector/Pool engines.
- exp(ST) is computed ONCE into 128 bf16 [128,512] tiles; the main
  sweep is a flat 128-iteration software pipeline (scores run 2
  iterations ahead) pacing PE/ACT/DVE in lockstep at ~870ns/iter.
  Row-softmax epilogues run on the Pool engine; reciprocals use the
  fast approx DVE op.
- The 16KB pairwise AllReduce of column sums is split in two: the
  first half launches 16 iterations before sweep end, the second half
  hides behind path-1's first 16 m-tiles (all four n-blocks' PSUM
  accumulations held open across 8 banks).
- Path 1 (column softmax) is a pure matmul sweep over the stored est
  tiles with pre-scaled Z2.

Shapes (hardcoded): x [4,256,64,64] f32 -> out [4,512,64,64] f32.
"""
import numpy as np

import concourse.bacc as bacc
import concourse.mybir as mybir
import concourse.tile as tile
from concourse.bass_utils import run_bass_kernel_spmd

B, C, H, W = 4, 256, 64, 64
N = H * W            # 4096 pixels / score dim
NH = N // 2          # 2048 local score rows per core
CK = C // 128        # 2 contraction chunks
MT = N // 128        # 32 m-tiles
NB = NH // 512       # 4 n-blocks of 512
T = 16               # N = 16*C interleave factor for the .view trick
SHIFT = 64.0         # constant softmax shift (randn logits ~ N(0, 16^2))
TOT = NB * MT        # 128 flat sweep iterations
HALF = MT // 2       # AllReduce split point (m-tiles per half)

F32 = mybir.dt.float32
F32R = mybir.dt.float32r
F16 = mybir.dt.float16
BF16 = mybir.dt.bfloat16
ADD = mybir.AluOpType.add
MULT = mybir.AluOpType.mult
MAX = mybir.AluOpType.max
IDENT = mybir.ActivationFunctionType.Identity
EXP = mybir.ActivationFunctionType.Exp

_CACHE = {}


def _build_nc():
    nc = bacc.Bacc("TRN2", target_bir_lowering=False, debug=False, num_devices=8)

    x_full_d = nc.dram_tensor("x_full", [C, N], F16, kind="ExternalInput")
    x_half_d = nc.dram_tensor("x_half", [C, NH], F32, kind="ExternalInput")
    wtf_d = nc.dram_tensor("wtf", [C, C], F16, kind="ExternalInput")
    # wtg13: concat(roll(w_teta.T)[:, :128], w_gi.T) -> [C, 384]
    wtg13_d = nc.dram_tensor("wtg13", [C, 384], F16, kind="ExternalInput")
    # wo: concat(w_o1.T, w_o2.T) along columns -> [C, 2C]
    wo_d = nc.dram_tensor("wo", [C, 2 * C], F16, kind="ExternalInput")
    bt_d = nc.dram_tensor("bt", [1, 128], F32, kind="ExternalInput")
    bg_d = nc.dram_tensor("bg", [1, C], F32, kind="ExternalInput")
    bf_d = nc.dram_tensor("bf", [128, 2], F32, kind="ExternalInput")
    bo1_d = nc.dram_tensor("bo1", [128, 2], F32, kind="ExternalInput")
    bo2_d = nc.dram_tensor("bo2", [128, 2], F32, kind="ExternalInput")
    out_d = nc.dram_tensor("out", [2 * C, NH], F32, kind="ExternalOutput")

    with tile.TileContext(nc) as tc:
        with (
            tc.tile_pool(name="res", bufs=1) as res,
            tc.tile_pool(name="dram", bufs=1, space="DRAM") as dram,
        ):
            # ---------------- resident tiles ----------------
            X1vT = [res.tile([128, NH], F16, name=f"x1vt{k}") for k in range(CK)]
            X2 = [res.tile([128, N], F16, name=f"x2_{k}") for k in range(CK)]
            Z1T = res.tile([128, MT * 256], BF16, name="Z1T")
            Z2T = [res.tile([128, 256], BF16, name=f"z2t{j}") for j in range(MT)]
            ones_f32 = res.tile([128, 128], F32, name="ones_f32")
            nc.vector.memset(ones_f32[:], 1.0)
            ones_bf = res.tile([128, 128], BF16, name="ones_bf")
            nc.vector.memset(ones_bf[:], 1.0)
            bf_sb = res.tile([128, 2], F32, name="bf_sb")
            bo1_sb = res.tile([128, 2], F32, name="bo1_sb")
            bo2_sb = res.tile([128, 2], F32, name="bo2_sb")
            nc.sync.dma_start(bf_sb[:], bf_d[:, :])
            nc.sync.dma_start(bo1_sb[:], bo1_d[:, :])
            nc.sync.dma_start(bo2_sb[:], bo2_d[:, :])
            neg_shift = res.tile([128, 1], F32, name="neg_shift")
            nc.vector.memset(neg_shift[:], -SHIFT)
            colsumP = res.tile([128, MT * NB], F32, name="colsumP")
            colscale = [res.tile([128, HALF], F32, name=f"colscale{h}")
                        for h in range(2)]
            btrep = res.tile([128, 128], F32, name="btrep")
            bgrep = res.tile([128, C], F32, name="bgrep")

            # ---------------- phase P: loads + projections + Z ----------
            with tc.tile_pool(name="px", bufs=1) as px:
                bst = px.tile([1, 128], F32, name="bst")
                nc.sync.dma_start(bst[:], bt_d[:, :])
                bst2 = px.tile([1, C], F32, name="bst2")
                nc.sync.dma_start(bst2[:], bg_d[:, :])

                wtf_s = [px.tile([128, C], F16, name=f"wtf{k}") for k in range(CK)]
                wtg13_s = [px.tile([128, 384], F16, name=f"wtg13{k}")
                           for k in range(CK)]
                wo_s = [px.tile([128, 2 * C], F16, name=f"wo{k}") for k in range(CK)]
                x_s = [px.tile([128, N], F16, name=f"xs{k}") for k in range(CK)]
                for k in range(CK):
                    nc.sync.dma_start(wtf_s[k][:], wtf_d[128 * k:128 * (k + 1), :])
                    nc.sync.dma_start(
                        wtg13_s[k][:], wtg13_d[128 * k:128 * (k + 1), :])
                    nc.sync.dma_start(wo_s[k][:], wo_d[128 * k:128 * (k + 1), :])
                # x in 512-column chunks so compute starts early
                for j in range(N // 512):
                    for k in range(CK):
                        nc.sync.dma_start(
                            x_s[k][:, 512 * j:512 * (j + 1)],
                            x_full_d[128 * k:128 * (k + 1),
                                     512 * j:512 * (j + 1)])

                X3vT = [px.tile([128, N], F16, name=f"x3vt{k}") for k in range(CK)]

                with tc.tile_pool(name="pp1", bufs=1, space="PSUM") as pp1:
                    # replicated free-dim bias rows via ones-matmul
                    pbr = pp1.tile([128, C], F32, tag="pbr", bufs=1, name="pbr")
                    nc.tensor.matmul(pbr[:, 0:128], ones_f32[0:1, :], bst[0:1, :],
                                     start=True, stop=True)
                    nc.vector.tensor_copy(btrep[:], pbr[:, 0:128])
                    pbr2 = pp1.tile([128, C], F32, tag="pbr", bufs=1, name="pbr2")
                    nc.tensor.matmul(pbr2[:], ones_f32[0:1, :], bst2[0:1, :],
                                     start=True, stop=True)
                    nc.vector.tensor_copy(bgrep[:], pbr2[:])

                    # interleave the X2 and X1/X3 streams: per step one
                    # X2 tile (ACT writes) + two p13 tiles (DVE writes
                    # X1v, Pool writes X3v) -> PE stays busy, consumers
                    # spread over three engines
                    x1v_v = [X1vT[k].rearrange("p (q t) -> p q t", t=T)
                             for k in range(CK)]
                    x3v_v = [X3vT[k].rearrange("p (q t) -> p q t", t=T)
                             for k in range(CK)]
                    for s in range(16):
                        j, i = s // 2, s % 2
                        p2 = pp1.tile([128, 512], F32, tag="p2", bufs=2,
                                      name=f"p2_{i}_{j}")
                        for k in range(CK):
                            nc.tensor.matmul(
                                p2[:], wtf_s[k][:, 128 * i:128 * (i + 1)],
                                x_s[k][:, 512 * j:512 * (j + 1)],
                                start=(k == 0), stop=(k == CK - 1),
                            )
                        nc.scalar.activation(
                            X2[i][:, 512 * j:512 * (j + 1)], p2[:], IDENT,
                            bias=bf_sb[:, i:i + 1],
                        )
                        t = s
                        for ci in range(2):
                            p13 = pp1.tile([128, 384], F32, tag="p13", bufs=3,
                                           name=f"p13_{t}_{ci}")
                            for k in range(CK):
                                nc.tensor.matmul(
                                    p13[:],
                                    x_s[k][:, 256 * t + 128 * ci:
                                           256 * t + 128 * (ci + 1)],
                                    wtg13_s[k][:],
                                    start=(k == 0), stop=(k == CK - 1),
                                )
                            nc.vector.tensor_tensor(
                                x1v_v[ci][:, :, t], p13[:, 0:128], btrep[:], ADD)
                            nc.vector.tensor_tensor(
                                x3v_v[ci][:, :, t], p13[:, 128:384], bgrep[:], ADD)

                # ---------------- Z build ----------------
                with tc.tile_pool(name="pzp", bufs=3, space="PSUM") as pzp:
                    for j in range(MT):
                        pzt = pzp.tile([128, 512], F32, tag="pzt", name=f"pzt{j}")
                        for k in range(CK):
                            nc.tensor.matmul(
                                pzt[:], X3vT[k][:, 128 * j:128 * (j + 1)],
                                wo_s[k][:],
                                start=(k == 0), stop=(k == CK - 1),
                            )
                        if j % 2 == 0:
                            nc.vector.tensor_copy(
                                Z1T[:, 256 * j:256 * (j + 1)], pzt[:, 0:256])
                            nc.scalar.activation(Z2T[j][:], pzt[:, 256:512], IDENT)
                        else:
                            nc.scalar.activation(
                                Z1T[:, 256 * j:256 * (j + 1)], pzt[:, 0:256],
                                IDENT)
                            nc.vector.tensor_copy(Z2T[j][:], pzt[:, 256:512])

            # ---------------- main sweep + collectives + path1 ----------
            with (
                tc.tile_pool(name="estp", bufs=1) as estp,
                tc.tile_pool(name="sw", bufs=1) as sw,
            ):
                est_t = [estp.tile([128, 512], BF16, name=f"est_{ix}")
                         for ix in range(TOT)]
                cl_t = [sw.tile([128, HALF], F32, name=f"cl{h}") for h in range(2)]
                cg_t = [sw.tile([128, HALF], F32, name=f"cg{h}") for h in range(2)]
                ar_in = [dram.tile([128, HALF], F32, name=f"ar_in{h}")
                         for h in range(2)]
                ar_out = [dram.tile([128, HALF], F32, name=f"ar_out{h}")
                          for h in range(2)]
                csview = colsumP.rearrange("p (m b) -> p m b", b=NB)

                def launch_ar(h):
                    # local colsum reduce for this half (Pool), stage to
                    # DRAM (SP queue), trigger the pairwise AllReduce
                    # (Pool; trigger only, CC engine does the work)
                    nc.vector.tensor_reduce(
                        cl_t[h][:], csview[:, HALF * h:HALF * (h + 1), :],
                        axis=mybir.AxisListType.X, op=ADD)
                    nc.sync.dma_start(ar_in[h][:], cl_t[h][:])
                    nc.gpsimd.collective_compute(
                        "AllReduce", ADD,
                        replica_groups=[[0, 1], [2, 3], [4, 5], [6, 7]],
                        ins=[ar_in[h].opt()], outs=[ar_out[h].opt()],
                    )

                with (
                    tc.tile_pool(name="pstp", bufs=3, space="PSUM") as pstp,
                    tc.tile_pool(name="paccp", bufs=2, space="PSUM") as paccp,
                    tc.tile_pool(name="prsp", bufs=1, space="PSUM") as prsp,
                ):
                    pst_tiles = {}
                    po_nb = {}
                    rowacc_nb = {}
                    pending = [None]

                    def issue_pst(ix):
                        nb, mj = divmod(ix, MT)
                        p = pstp.tile([128, 512], F32, tag="st", name=f"pst_{ix}")
                        for k in range(CK):
                            nc.tensor.matmul(
                                p[:], X2[k][:, 128 * mj:128 * (mj + 1)],
                                X1vT[k][:, 512 * nb:512 * (nb + 1)],
                                start=(k == 0), stop=(k == CK - 1),
                            )
                        pst_tiles[ix] = p

                    def issue_est(ix):
                        nb, mj = divmod(ix, MT)
                        col = NB * mj + nb
                        nc.scalar.activation(
                            est_t[ix][:], pst_tiles.pop(ix)[:], EXP,
                            bias=neg_shift[:],
                            accum_out=colsumP[:, col:col + 1],
                        )

                    def epilogue0(nb):
                        # deferred rowsum + path0 epilogue for block nb:
                        # prs on PE (after two fresh psts), fast-approx
                        # reciprocal on DVE, elementwise tail on Pool
                        prs = prsp.tile([128, 512], F32, tag="rs",
                                        name=f"prs{nb}")
                        nc.tensor.matmul(prs[:], ones_bf[:], rowacc_nb[nb][:],
                                         start=True, stop=True)
                        rrep = sw.tile([128, 512], F32, tag="rrep", bufs=2,
                                       name=f"rrep{nb}")
                        nc.vector.reciprocal_approx_fast(rrep[:], prs[:])
                        po = po_nb.pop(nb)
                        for i in range(2):
                            xt = sw.tile([128, 512], F32, tag="xt", bufs=2,
                                         name=f"xt0_{nb}_{i}")
                            nc.sync.dma_start(
                                xt[:], x_half_d[128 * i:128 * (i + 1),
                                                512 * nb:512 * (nb + 1)])
                            on = sw.tile([128, 512], F32, tag="on", bufs=2,
                                         name=f"on0_{nb}_{i}")
                            nc.vector.tensor_tensor(on[:], po[i][:], rrep[:], MULT)
                            nc.vector.tensor_tensor(on[:], on[:], xt[:], ADD)
                            nc.vector.tensor_scalar(
                                on[:], on[:], bo1_sb[:, i:i + 1], 0.0, ADD, MAX)
                            nc.sync.dma_start(
                                out_d[128 * i:128 * (i + 1),
                                      512 * nb:512 * (nb + 1)], on[:])

                    issue_pst(0)
                    issue_est(0)
                    issue_pst(1)
                    issue_est(1)
                    for ix in range(TOT):
                        nb, mj = divmod(ix, MT)
                        if ix + 2 < TOT:
                            issue_pst(ix + 2)
                            issue_est(ix + 2)
                        if mj == 0:
                            po_nb[nb] = [
                                paccp.tile([128, 512], F32, tag=f"po{i}", bufs=2,
                                           name=f"po{i}_{nb}") for i in range(2)]
                            rowacc_nb[nb] = sw.tile(
                                [128, 512], BF16, tag="rowacc", bufs=2,
                                name=f"rowacc{nb}")
                            nc.vector.tensor_copy(rowacc_nb[nb][:], est_t[ix][:])
                        else:
                            nc.vector.tensor_tensor(
                                rowacc_nb[nb][:], rowacc_nb[nb][:],
                                est_t[ix][:], ADD)
                        if mj == 1 and pending[0] is not None:
                            pending[0]()
                            pending[0] = None
                        if ix == TOT - (MT - HALF) + 1:
                            launch_ar(0)
                        for i in range(2):
                            nc.tensor.matmul(
                                po_nb[nb][i][:],
                                Z1T[:, 256 * mj + 128 * i:
                                    256 * mj + 128 * (i + 1)],
                                est_t[ix][:],
                                start=(mj == 0), stop=(mj == MT - 1),
                            )
                        if mj == MT - 1:
                            pending[0] = (lambda nb=nb: epilogue0(nb))

                    # second-half AllReduce, then the deferred last
                    # epilogue overlaps its latency
                    launch_ar(1)
                    # cg DMA-backs on the Pool queue: AR1 is done (or
                    # nearly) by now, so the wait doesn't block the
                    # Z2 scales queued behind it
                    nc.gpsimd.dma_start(cg_t[0][:], ar_out[0][:])
                    nc.vector.reciprocal_approx_fast(colscale[0][:], cg_t[0][:])
                    for j in range(HALF):
                        nc.gpsimd.tensor_scalar_mul(
                            Z2T[j][:], Z2T[j][:], colscale[0][:, j:j + 1])
                    pending[0]()
                    pending[0] = None
                    nc.gpsimd.dma_start(cg_t[1][:], ar_out[1][:])
                    nc.vector.reciprocal_approx_fast(colscale[1][:], cg_t[1][:])
                    for j in range(HALF, MT):
                        nc.gpsimd.tensor_scalar_mul(
                            Z2T[j][:], Z2T[j][:],
                            colscale[1][:, j - HALF:j - HALF + 1])

                # ---------------- path 1: pure matmul sweep --------------
                # all four n-blocks' accumulations stay open across the 8
                # PSUM banks; mj 0..15 runs while AR2 is still in flight
                with tc.tile_pool(name="pacc2", bufs=1, space="PSUM") as pacc2:
                    po1 = [[pacc2.tile([128, 512], F32, tag=f"q{nb}_{i}", bufs=1,
                                       name=f"q{nb}_{i}") for i in range(2)]
                           for nb in range(NB)]
                    for mj in range(MT):
                        for i in range(2):
                            for nb in range(NB):
                                nc.tensor.matmul(
                                    po1[nb][i][:],
                                    Z2T[mj][:, 128 * i:128 * (i + 1)],
                                    est_t[MT * nb + mj][:],
                                    start=(mj == 0), stop=(mj == MT - 1),
                                )
                    for nb in range(NB):
                        for i in range(2):
                            xt = sw.tile([128, 512], F32, tag="xt", bufs=2,
                                         name=f"xt1_{nb}_{i}")
                            nc.sync.dma_start(
                                xt[:], x_half_d[128 * i:128 * (i + 1),
                                                512 * nb:512 * (nb + 1)])
                            on = sw.tile([128, 512], F32, tag="on", bufs=2,
                                         name=f"on1_{nb}_{i}")
                            nc.vector.tensor_tensor(
                                on[:], po1[nb][i][:], xt[:], ADD)
                            nc.vector.tensor_scalar(
                                on[:], on[:], bo2_sb[:, i:i + 1], 0.0, ADD, MAX)
                            nc.sync.dma_start(
                                out_d[C + 128 * i:C + 128 * (i + 1),
                                      512 * nb:512 * (nb + 1)], on[:])

    nc.compile()
    return nc


def _in_maps(x, w_teta, b_teta, w_fi, b_fi, w_gi, b_gi, w_o1, b_o1, w_o2, b_o2):
    xf = np.ascontiguousarray(x.reshape(B, C, N), dtype=np.float32)
    xf16 = xf.astype(np.float16)
    wtf = np.ascontiguousarray(w_fi.T, dtype=np.float16)
    wtgT = np.asarray(w_gi.T, dtype=np.float32)
    wo = np.ascontiguousarray(
        np.concatenate([w_o1.T, w_o2.T], axis=1), dtype=np.float16)
    bf = np.ascontiguousarray(b_fi.reshape(2, 128).T, dtype=np.float32)
    bo1 = np.ascontiguousarray(b_o1.reshape(2, 128).T, dtype=np.float32)
    bo2 = np.ascontiguousarray(b_o2.reshape(2, 128).T, dtype=np.float32)
    bg = np.ascontiguousarray(b_gi.reshape(1, C), dtype=np.float32)
    wtetaT = np.asarray(w_teta.T, dtype=np.float32)
    maps = []
    for c in range(8):
        b, r = c // 2, c % 2
        # rotate so the local q-half sits in columns 0:128, keep only it
        wtt_loc = np.roll(wtetaT, -128 * r, axis=1)[:, 0:128]
        wtg13 = np.ascontiguousarray(
            np.concatenate([wtt_loc, wtgT], axis=1), dtype=np.float16)
        maps.append({
            "x_full": xf16[b],
            "x_half": np.ascontiguousarray(xf[b][:, NH * r:NH * (r + 1)]),
            "wtg13": wtg13,
            "wtf": wtf, "wo": wo,
            "bt": np.ascontiguousarray(
                b_teta[128 * r:128 * (r + 1)].reshape(1, 128), dtype=np.float32),
            "bg": bg, "bf": bf, "bo1": bo1, "bo2": bo2,
        })
    return maps


def run(trace=False, **inputs):
    if "nc" not in _CACHE:
        _CACHE["nc"] = _build_nc()
    nc = _CACHE["nc"]
    maps = _in_maps(**inputs)
    res = run_bass_kernel_spmd(nc, maps, core_ids=list(range(8)), trace=trace)
    out = np.empty((B, 2 * C, N), dtype=np.float32)
    for c in range(8):
        b, r = c // 2, c % 2
        out[b][:, NH * r:NH * (r + 1)] = res.results[c]["out"]
    return out.reshape(B, 2 * C, H, W), res


def kernel(**inputs):
    out, _ = run(trace=False, **inputs)
    return out


# revision 36
# speedup vs baseline: 1.3605x; 1.3605x over previous
"""NonLocalBlock Trainium2 kernel (v3).

8-core split: data-parallel over batch B=4 (2 cores per batch element),
each core pair splits the [N,N] score matrix by rows n (core r owns
n in [2048r, 2048r+2048)). Scores are computed transposed (ST[m,n]) so
both output matmuls contract over m with m on partitions. The output
1x1 convs are folded in before the attention matmuls via
Z = (w_o @ X3v^T)^T, so no on-device transposes are needed anywhere.
Softmaxes use a constant shift (no per-row max): exp(s-64) is safe for
randn-scale inputs, and constant shifts cancel exactly in softmax.

Pipeline structure (all engines near-saturated):
- x / projection weights arrive fp16 (host-converted); all phase-P
  matmuls are fp16 at full PE rate with no staging copies.
- Phase P interleaves the X2 and X1/X3 projection streams so the PE
  stays continuously busy (ramps to 2.4 GHz); PSUM->SBUF copies are
  spread across Scalar/Vector/Pool engines.
- exp(ST) is computed ONCE into 128 bf16 [128,512] tiles; the main
  sweep is a flat 128-iteration software pipeline (scores run 2
  iterations ahead) pacing PE/ACT/DVE in lockstep at ~870ns/iter.
  Row-softmax epilogues run on the Pool engine; reciprocals use the
  fast approx DVE op.
- The 16KB pairwise AllReduce of column sums is split in two: the
  first half launches 16 iterations before sweep end, the second half
  hides behind path-1's first 16 m-tiles (all four n-blocks' PSUM
  accumulations held open across 8 banks).
- Path 1 (column softmax) is a pure matmul sweep over the stored est
  tiles with pre-scaled Z2.

Shapes (hardcoded): x [4,256,64,64] f32 -> out [4,512,64,64] f32.
"""
import numpy as np

import concourse.bacc as bacc
import concourse.mybir as mybir
import concourse.tile as tile
from concourse.bass_utils import run_bass_kernel_spmd

B, C, H, W = 4, 256, 64, 64
N = H * W            # 4096 pixels / score dim
NH = N // 2          # 2048 local score rows per core
CK = C // 128        # 2 contraction chunks
MT = N // 128        # 32 m-tiles
NB = NH // 512       # 4 n-blocks of 512
T = 16               # N = 16*C interleave factor for the .view trick
SHIFT = 64.0         # constant softmax shift (randn logits ~ N(0, 16^2))
TOT = NB * MT        # 128 flat sweep iterations
HALF = MT // 2       # AllReduce split point (m-tiles per half)

F32 = mybir.dt.float32
F32R = mybir.dt.float32r
F16 = mybir.dt.float16
BF16 = mybir.dt.bfloat16
ADD = mybir.AluOpType.add
MULT = mybir.AluOpType.mult
MAX = mybir.AluOpType.max
IDENT = mybir.ActivationFunctionType.Identity
EXP = mybir.ActivationFunctionType.Exp

_CACHE = {}


def _build_nc():
    nc = bacc.Bacc("TRN2", target_bir_lowering=False, debug=False, num_devices=8)

    x_full_d = nc.dram_tensor("x_full", [C, N], F16, kind="ExternalInput")
    # x with pixels in t-major (mpos) order, for the X2 projection: X2's
    # column order then matches X3vT's t-major chunk order, so every
    # m-tile slice downstream is contiguous
    x_mp_d = nc.dram_tensor("x_mp", [C, N], F16, kind="ExternalInput")
    x_half_d = nc.dram_tensor("x_half", [C, NH], F32, kind="ExternalInput")
    wtf_d = nc.dram_tensor("wtf", [C, C], F16, kind="ExternalInput")
    # wtg13: concat(roll(w_teta.T)[:, :128], w_gi.T) -> [C, 384]
    wtg13_d = nc.dram_tensor("wtg13", [C, 384], F16, kind="ExternalInput")
    # wo: concat(w_o1.T, w_o2.T) along columns -> [C, 2C]
    wo_d = nc.dram_tensor("wo", [C, 2 * C], F16, kind="ExternalInput")
    bt_d = nc.dram_tensor("bt", [1, 128], F32, kind="ExternalInput")
    bg_d = nc.dram_tensor("bg", [1, C], F32, kind="ExternalInput")
    bf_d = nc.dram_tensor("bf", [128, 2], F32, kind="ExternalInput")
    bo1_d = nc.dram_tensor("bo1", [128, 2], F32, kind="ExternalInput")
    bo2_d = nc.dram_tensor("bo2", [128, 2], F32, kind="ExternalInput")
    out_d = nc.dram_tensor("out", [2 * C, NH], F32, kind="ExternalOutput")

    with tile.TileContext(nc) as tc:
        with (
            tc.tile_pool(name="res", bufs=1) as res,
            tc.tile_pool(name="dram", bufs=1, space="DRAM") as dram,
        ):
            # ---------------- resident tiles ----------------
            X1vT = [res.tile([128, NH], F16, name=f"x1vt{k}") for k in range(CK)]
            X2 = [res.tile([128, N], F16, name=f"x2_{k}") for k in range(CK)]
            Z1T = res.tile([128, MT * 256], BF16, name="Z1T")
            Z2T = [res.tile([128, 256], BF16, name=f"z2t{j}") for j in range(MT)]
            ones_f32 = res.tile([128, 128], F32, name="ones_f32")
            nc.vector.memset(ones_f32[:], 1.0)
            ones_bf = res.tile([128, 128], BF16, name="ones_bf")
            nc.vector.memset(ones_bf[:], 1.0)
            bf_sb = res.tile([128, 2], F32, name="bf_sb")
            bo1_sb = res.tile([128, 2], F32, name="bo1_sb")
            bo2_sb = res.tile([128, 2], F32, name="bo2_sb")
            nc.sync.dma_start(bf_sb[:], bf_d[:, :])
            nc.sync.dma_start(bo1_sb[:], bo1_d[:, :])
            nc.sync.dma_start(bo2_sb[:], bo2_d[:, :])
            neg_shift = res.tile([128, 1], F32, name="neg_shift")
            nc.vector.memset(neg_shift[:], -SHIFT)
            colsumP = res.tile([128, MT * NB], F32, name="colsumP")
            colscale = [res.tile([128, HALF], F32, name=f"colscale{h}")
                        for h in range(2)]
            btrep = res.tile([128, 128], F32, name="btrep")
            bgrep = res.tile([128, C], F32, name="bgrep")

            # ---------------- phase P: loads + projections + Z ----------
            with tc.tile_pool(name="px", bufs=1) as px:
                bst = px.tile([1, 128], F32, name="bst")
                nc.sync.dma_start(bst[:], bt_d[:, :])
                bst2 = px.tile([1, C], F32, name="bst2")
                nc.sync.dma_start(bst2[:], bg_d[:, :])

                wtf_s = [px.tile([128, C], F16, name=f"wtf{k}") for k in range(CK)]
                wtg13_s = [px.tile([128, 384], F16, name=f"wtg13{k}")
                           for k in range(CK)]
                wo_s = [px.tile([128, 2 * C], F16, name=f"wo{k}") for k in range(CK)]
                x_s = [px.tile([128, N], F16, name=f"xs{k}") for k in range(CK)]
                xp_s = [px.tile([128, N], F16, name=f"xps{k}") for k in range(CK)]
                for k in range(CK):
                    nc.sync.dma_start(wtf_s[k][:], wtf_d[128 * k:128 * (k + 1), :])
                    nc.sync.dma_start(
                        wtg13_s[k][:], wtg13_d[128 * k:128 * (k + 1), :])
                    nc.sync.dma_start(wo_s[k][:], wo_d[128 * k:128 * (k + 1), :])
                # x in 512-column chunks so compute starts early
                for j in range(N // 512):
                    for k in range(CK):
                        nc.sync.dma_start(
                            x_s[k][:, 512 * j:512 * (j + 1)],
                            x_full_d[128 * k:128 * (k + 1),
                                     512 * j:512 * (j + 1)])
                        nc.sync.dma_start(
                            xp_s[k][:, 512 * j:512 * (j + 1)],
                            x_mp_d[128 * k:128 * (k + 1),
                                   512 * j:512 * (j + 1)])

                X3vT = [px.tile([128, N], F16, name=f"x3vt{k}") for k in range(CK)]

                with tc.tile_pool(name="pp1", bufs=1, space="PSUM") as pp1:
                    # replicated free-dim bias rows via ones-matmul
                    pbr = pp1.tile([128, C], F32, tag="pbr", bufs=1, name="pbr")
                    nc.tensor.matmul(pbr[:, 0:128], ones_f32[0:1, :], bst[0:1, :],
                                     start=True, stop=True)
                    nc.vector.tensor_copy(btrep[:], pbr[:, 0:128])
                    pbr2 = pp1.tile([128, C], F32, tag="pbr", bufs=1, name="pbr2")
                    nc.tensor.matmul(pbr2[:], ones_f32[0:1, :], bst2[0:1, :],
                                     start=True, stop=True)
                    nc.vector.tensor_copy(bgrep[:], pbr2[:])

                    # interleave the X2 and X1/X3 streams: per step one
                    # X2 tile (ACT writes) + two p13 tiles (DVE writes).
                    # X1vT/X3vT are stored t-major ([c, 128t+q]) so every
                    # projection write is CONTIGUOUS (strided SBUF writes
                    # run at ~4-8 cycles/elem on the DVE); the consumers
                    # read them through strided APs instead, which the PE
                    # streams at full rate.
                    for s in range(16):
                        j, i = s // 2, s % 2
                        p2 = pp1.tile([128, 512], F32, tag="p2", bufs=2,
                                      name=f"p2_{i}_{j}")
                        for k in range(CK):
                            nc.tensor.matmul(
                                p2[:], wtf_s[k][:, 128 * i:128 * (i + 1)],
                                xp_s[k][:, 512 * j:512 * (j + 1)],
                                start=(k == 0), stop=(k == CK - 1),
                            )
                        nc.scalar.activation(
                            X2[i][:, 512 * j:512 * (j + 1)], p2[:], IDENT,
                            bias=bf_sb[:, i:i + 1],
                        )
                        t = s
                        for ci in range(2):
                            p13 = pp1.tile([128, 384], F32, tag="p13", bufs=3,
                                           name=f"p13_{t}_{ci}")
                            for k in range(CK):
                                nc.tensor.matmul(
                                    p13[:],
                                    x_s[k][:, 256 * t + 128 * ci:
                                           256 * t + 128 * (ci + 1)],
                                    wtg13_s[k][:],
                                    start=(k == 0), stop=(k == CK - 1),
                                )
                            nc.vector.tensor_tensor(
                                X1vT[ci][:, 128 * t:128 * (t + 1)],
                                p13[:, 0:128], btrep[:], ADD)
                            nc.vector.tensor_tensor(
                                X3vT[ci][:, 256 * t:256 * (t + 1)],
                                p13[:, 128:384], bgrep[:], ADD)

                # ---------------- Z build ----------------
                # X3vT is t-major (mpos order), matching X2's column
                # order, so plain contiguous chunks line up with the
                # est m-tiles
                with tc.tile_pool(name="pzp", bufs=3, space="PSUM") as pzp:
                    for j in range(MT):
                        pzt = pzp.tile([128, 512], F32, tag="pzt", name=f"pzt{j}")
                        for k in range(CK):
                            nc.tensor.matmul(
                                pzt[:], X3vT[k][:, 128 * j:128 * (j + 1)],
                                wo_s[k][:],
                                start=(k == 0), stop=(k == CK - 1),
                            )
                        if j % 2 == 0:
                            nc.vector.tensor_copy(
                                Z1T[:, 256 * j:256 * (j + 1)], pzt[:, 0:256])
                            nc.scalar.activation(Z2T[j][:], pzt[:, 256:512], IDENT)
                        else:
                            nc.scalar.activation(
                                Z1T[:, 256 * j:256 * (j + 1)], pzt[:, 0:256],
                                IDENT)
                            nc.vector.tensor_copy(Z2T[j][:], pzt[:, 256:512])

            # ---------------- main sweep + collectives + path1 ----------
            with (
                tc.tile_pool(name="estp", bufs=1) as estp,
                tc.tile_pool(name="sw", bufs=1) as sw,
            ):
                est_t = [estp.tile([128, 512], BF16, name=f"est_{ix}")
                         for ix in range(TOT)]
                cl_t = [sw.tile([128, HALF], F32, name=f"cl{h}") for h in range(2)]
                cg_t = [sw.tile([128, HALF], F32, name=f"cg{h}") for h in range(2)]
                ar_in = [dram.tile([128, HALF], F32, name=f"ar_in{h}")
                         for h in range(2)]
                ar_out = [dram.tile([128, HALF], F32, name=f"ar_out{h}")
                          for h in range(2)]
                csview = colsumP.rearrange("p (m b) -> p m b", b=NB)

                def launch_ar(h):
                    # local colsum reduce for this half (Pool), stage to
                    # DRAM (SP queue), trigger the pairwise AllReduce
                    # (Pool; trigger only, CC engine does the work)
                    nc.vector.tensor_reduce(
                        cl_t[h][:], csview[:, HALF * h:HALF * (h + 1), :],
                        axis=mybir.AxisListType.X, op=ADD)
                    nc.sync.dma_start(ar_in[h][:], cl_t[h][:])
                    nc.gpsimd.collective_compute(
                        "AllReduce", ADD,
                        replica_groups=[[0, 1], [2, 3], [4, 5], [6, 7]],
                        ins=[ar_in[h].opt()], outs=[ar_out[h].opt()],
                    )

                with (
                    tc.tile_pool(name="pstp", bufs=3, space="PSUM") as pstp,
                    tc.tile_pool(name="paccp", bufs=2, space="PSUM") as paccp,
                    tc.tile_pool(name="prsp", bufs=1, space="PSUM") as prsp,
                ):
                    pst_tiles = {}
                    po_nb = {}
                    rowacc_nb = {}
                    pending = [None]

                    # X1vT is t-major, so contiguous 512-blocks cover a
                    # PERMUTED set of n (pos = 128t + q <-> n = 16q + t).
                    # The whole sweep runs in pos-order; the host permutes
                    # x_half on the way in and un-permutes out.
                    def issue_pst(ix):
                        nb, mj = divmod(ix, MT)
                        p = pstp.tile([128, 512], F32, tag="st", name=f"pst_{ix}")
                        for k in range(CK):
                            nc.tensor.matmul(
                                p[:], X2[k][:, 128 * mj:128 * (mj + 1)],
                                X1vT[k][:, 512 * nb:512 * (nb + 1)],
                                start=(k == 0), stop=(k == CK - 1),
                            )
                        pst_tiles[ix] = p

                    def issue_est(ix):
                        nb, mj = divmod(ix, MT)
                        col = NB * mj + nb
                        nc.scalar.activation(
                            est_t[ix][:], pst_tiles.pop(ix)[:], EXP,
                            bias=neg_shift[:],
                            accum_out=colsumP[:, col:col + 1],
                        )

                    def epilogue0(nb):
                        # deferred rowsum + path0 epilogue for block nb:
                        # prs on PE (after two fresh psts), fast-approx
                        # reciprocal on DVE, elementwise tail on Pool
                        prs = prsp.tile([128, 512], F32, tag="rs",
                                        name=f"prs{nb}")
                        nc.tensor.matmul(prs[:], ones_bf[:], rowacc_nb[nb][:],
                                         start=True, stop=True)
                        rrep = sw.tile([128, 512], F32, tag="rrep", bufs=2,
                                       name=f"rrep{nb}")
                        nc.vector.reciprocal_approx_fast(rrep[:], prs[:])
                        po = po_nb.pop(nb)
                        for i in range(2):
                            xt = sw.tile([128, 512], F32, tag="xt", bufs=2,
                                         name=f"xt0_{nb}_{i}")
                            nc.sync.dma_start(
                                xt[:], x_half_d[128 * i:128 * (i + 1),
                                                512 * nb:512 * (nb + 1)])
                            on = sw.tile([128, 512], F32, tag="on", bufs=2,
                                         name=f"on0_{nb}_{i}")
                            nc.vector.tensor_tensor(on[:], po[i][:], rrep[:], MULT)
                            nc.vector.tensor_tensor(on[:], on[:], xt[:], ADD)
                            nc.vector.tensor_scalar(
                                on[:], on[:], bo1_sb[:, i:i + 1], 0.0, ADD, MAX)
                            nc.sync.dma_start(
                                out_d[128 * i:128 * (i + 1),
                                      512 * nb:512 * (nb + 1)], on[:])

                    issue_pst(0)
                    issue_est(0)
                    issue_pst(1)
                    issue_est(1)
                    for ix in range(TOT):
                        nb, mj = divmod(ix, MT)
                        if ix + 2 < TOT:
                            issue_pst(ix + 2)
                            issue_est(ix + 2)
                        if mj == 0:
                            po_nb[nb] = [
                                paccp.tile([128, 512], F32, tag=f"po{i}", bufs=2,
                                           name=f"po{i}_{nb}") for i in range(2)]
                            rowacc_nb[nb] = sw.tile(
                                [128, 512], BF16, tag="rowacc", bufs=2,
                                name=f"rowacc{nb}")
                            nc.vector.tensor_copy(rowacc_nb[nb][:], est_t[ix][:])
                        else:
                            nc.vector.tensor_tensor(
                                rowacc_nb[nb][:], rowacc_nb[nb][:],
                                est_t[ix][:], ADD)
                        if mj == 1 and pending[0] is not None:
                            pending[0]()
                            pending[0] = None
                        if ix == TOT - (MT - HALF) + 1:
                            launch_ar(0)
                        for i in range(2):
                            nc.tensor.matmul(
                                po_nb[nb][i][:],
                                Z1T[:, 256 * mj + 128 * i:
                                    256 * mj + 128 * (i + 1)],
                                est_t[ix][:],
                                start=(mj == 0), stop=(mj == MT - 1),
                            )
                        if mj == MT - 1:
                            pending[0] = (lambda nb=nb: epilogue0(nb))

                    # second-half AllReduce, then the deferred last
                    # epilogue overlaps its latency
                    launch_ar(1)
                    # cg DMA-backs on the Pool queue: AR1 is done (or
                    # nearly) by now, so the wait doesn't block the
                    # Z2 scales queued behind it
                    nc.gpsimd.dma_start(cg_t[0][:], ar_out[0][:])
                    nc.vector.reciprocal_approx_fast(colscale[0][:], cg_t[0][:])
                    for j in range(HALF):
                        nc.vector.tensor_scalar_mul(
                            Z2T[j][:], Z2T[j][:], colscale[0][:, j:j + 1])
                    pending[0]()
                    pending[0] = None
                    nc.gpsimd.dma_start(cg_t[1][:], ar_out[1][:])
                    nc.vector.reciprocal_approx_fast(colscale[1][:], cg_t[1][:])
                    for j in range(HALF, MT):
                        nc.vector.tensor_scalar_mul(
                            Z2T[j][:], Z2T[j][:],
                            colscale[1][:, j - HALF:j - HALF + 1])

                # ---------------- path 1: pure matmul sweep --------------
                # all four n-blocks' accumulations stay open across the 8
                # PSUM banks; mj 0..15 runs while AR2 is still in flight
                with tc.tile_pool(name="pacc2", bufs=1, space="PSUM") as pacc2:
                    po1 = [[pacc2.tile([128, 512], F32, tag=f"q{nb}_{i}", bufs=1,
                                       name=f"q{nb}_{i}") for i in range(2)]
                           for nb in range(NB)]
                    for mj in range(MT):
                        for i in range(2):
                            for nb in range(NB):
                                nc.tensor.matmul(
                                    po1[nb][i][:],
                                    Z2T[mj][:, 128 * i:128 * (i + 1)],
                                    est_t[MT * nb + mj][:],
                                    start=(mj == 0), stop=(mj == MT - 1),
                                )
                    for nb in range(NB):
                        for i in range(2):
                            xt = sw.tile([128, 512], F32, tag="xt", bufs=2,
                                         name=f"xt1_{nb}_{i}")
                            nc.sync.dma_start(
                                xt[:], x_half_d[128 * i:128 * (i + 1),
                                                512 * nb:512 * (nb + 1)])
                            on = sw.tile([128, 512], F32, tag="on", bufs=2,
                                         name=f"on1_{nb}_{i}")
                            nc.vector.tensor_tensor(
                                on[:], po1[nb][i][:], xt[:], ADD)
                            nc.vector.tensor_scalar(
                                on[:], on[:], bo2_sb[:, i:i + 1], 0.0, ADD, MAX)
                            nc.sync.dma_start(
                                out_d[C + 128 * i:C + 128 * (i + 1),
                                      512 * nb:512 * (nb + 1)], on[:])

    nc.compile()
    return nc


# n-axis permutation of the device's local pixel columns: device column
# pos holds natural local pixel n = 16*(pos%128) + pos//128 (t-major
# X1vT storage). x_half is permuted on input, out un-permuted on gather.
_PERM = 16 * (np.arange(NH) % 128) + np.arange(NH) // 128
# m-axis (full pixel range) t-major permutation for the X2 input copy
_PERM_M = 16 * (np.arange(N) % 256) + np.arange(N) // 256


def _in_maps(x, w_teta, b_teta, w_fi, b_fi, w_gi, b_gi, w_o1, b_o1, w_o2, b_o2):
    xf = np.ascontiguousarray(x.reshape(B, C, N), dtype=np.float32)
    xf16 = xf.astype(np.float16)
    wtf = np.ascontiguousarray(w_fi.T, dtype=np.float16)
    wtgT = np.asarray(w_gi.T, dtype=np.float32)
    wo = np.ascontiguousarray(
        np.concatenate([w_o1.T, w_o2.T], axis=1), dtype=np.float16)
    bf = np.ascontiguousarray(b_fi.reshape(2, 128).T, dtype=np.float32)
    bo1 = np.ascontiguousarray(b_o1.reshape(2, 128).T, dtype=np.float32)
    bo2 = np.ascontiguousarray(b_o2.reshape(2, 128).T, dtype=np.float32)
    bg = np.ascontiguousarray(b_gi.reshape(1, C), dtype=np.float32)
    wtetaT = np.asarray(w_teta.T, dtype=np.float32)
    maps = []
    for c in range(8):
        b, r = c // 2, c % 2
        # rotate so the local q-half sits in columns 0:128, keep only it
        wtt_loc = np.roll(wtetaT, -128 * r, axis=1)[:, 0:128]
        wtg13 = np.ascontiguousarray(
            np.concatenate([wtt_loc, wtgT], axis=1), dtype=np.float16)
        maps.append({
            "x_full": xf16[b],
            "x_mp": np.ascontiguousarray(xf16[b][:, _PERM_M]),
            "x_half": np.ascontiguousarray(
                xf[b][:, NH * r:NH * (r + 1)][:, _PERM]),
            "wtg13": wtg13,
            "wtf": wtf, "wo": wo,
            "bt": np.ascontiguousarray(
                b_teta[128 * r:128 * (r + 1)].reshape(1, 128), dtype=np.float32),
            "bg": bg, "bf": bf, "bo1": bo1, "bo2": bo2,
        })
    return maps


def run(trace=False, **inputs):
    if "nc" not in _CACHE:
        _CACHE["nc"] = _build_nc()
    nc = _CACHE["nc"]
    maps = _in_maps(**inputs)
    res = run_bass_kernel_spmd(nc, maps, core_ids=list(range(8)), trace=trace)
    out = np.empty((B, 2 * C, N), dtype=np.float32)
    for c in range(8):
        b, r = c // 2, c % 2
        blk = out[b][:, NH * r:NH * (r + 1)]
        blk[:, _PERM] = res.results[c]["out"]
    return out.reshape(B, 2 * C, H, W), res


def kernel(**inputs):
    out, _ = run(trace=False, **inputs)
    return out


# revision 42
# speedup vs baseline: 1.4819x; 1.0893x over previous
"""NonLocalBlock Trainium2 kernel (v3).

8-core split: data-parallel over batch B=4 (2 cores per batch element),
each core pair splits the [N,N] score matrix by rows n (core r owns
n in [2048r, 2048r+2048)). Scores are computed transposed (ST[m,n]) so
both output matmuls contract over m with m on partitions. The output
1x1 convs are folded in before the attention matmuls via
Z = (w_o @ X3v^T)^T, so no on-device transposes are needed anywhere.
Softmaxes use a constant shift (no per-row max): exp(s-64) is safe for
randn-scale inputs, and constant shifts cancel exactly in softmax.

Pipeline structure (all engines near-saturated):
- x / projection weights arrive fp16 (host-converted); all phase-P
  matmuls are fp16 at full PE rate with no staging copies.
- Phase P interleaves the X2 and X1/X3 projection streams so the PE
  stays continuously busy (ramps to 2.4 GHz); PSUM->SBUF copies are
  spread across Scalar/Vector/Pool engines.
- exp(ST) is computed ONCE into 128 bf16 [128,512] tiles; the main
  sweep is a flat 128-iteration software pipeline (scores run 2
  iterations ahead) pacing PE/ACT/DVE in lockstep at ~870ns/iter.
  Row-softmax epilogues run on the Pool engine; reciprocals use the
  fast approx DVE op.
- The 16KB pairwise AllReduce of column sums is split in two: the
  first half launches 16 iterations before sweep end, the second half
  hides behind path-1's first 16 m-tiles (all four n-blocks' PSUM
  accumulations held open across 8 banks).
- Path 1 (column softmax) is a pure matmul sweep over the stored est
  tiles with pre-scaled Z2.

Shapes (hardcoded): x [4,256,64,64] f32 -> out [4,512,64,64] f32.
"""
import numpy as np

import concourse.bacc as bacc
import concourse.mybir as mybir
import concourse.tile as tile
from concourse.bass_utils import run_bass_kernel_spmd

B, C, H, W = 4, 256, 64, 64
N = H * W            # 4096 pixels / score dim
NH = N // 2          # 2048 local score rows per core
CK = C // 128        # 2 contraction chunks
MT = N // 128        # 32 m-tiles
NB = NH // 512       # 4 n-blocks of 512
T = 16               # N = 16*C interleave factor for the .view trick
SHIFT = 64.0         # constant softmax shift (randn logits ~ N(0, 16^2))
TOT = NB * MT        # 128 flat sweep iterations
HALF = MT // 2       # AllReduce split point (m-tiles per half)

F32 = mybir.dt.float32
F32R = mybir.dt.float32r
F16 = mybir.dt.float16
BF16 = mybir.dt.bfloat16
ADD = mybir.AluOpType.add
MULT = mybir.AluOpType.mult
MAX = mybir.AluOpType.max
IDENT = mybir.ActivationFunctionType.Identity
EXP = mybir.ActivationFunctionType.Exp

_CACHE = {}


def _build_nc():
    nc = bacc.Bacc("TRN2", target_bir_lowering=False, debug=False, num_devices=8)

    x_full_d = nc.dram_tensor("x_full", [C, N], F16, kind="ExternalInput")
    # x with pixels in t-major (mpos) order, for the X2 projection: X2's
    # column order then matches X3vT's t-major chunk order, so every
    # m-tile slice downstream is contiguous
    x_mp_d = nc.dram_tensor("x_mp", [C, N], F16, kind="ExternalInput")
    x_half_d = nc.dram_tensor("x_half", [C, NH], F32, kind="ExternalInput")
    wtf_d = nc.dram_tensor("wtf", [C, C], F16, kind="ExternalInput")
    # wtg13: concat(roll(w_teta.T)[:, :128], w_gi.T) -> [C, 384]
    wtg13_d = nc.dram_tensor("wtg13", [C, 384], F16, kind="ExternalInput")
    # wo: concat(w_o1.T, w_o2.T) along columns -> [C, 2C]
    wo_d = nc.dram_tensor("wo", [C, 2 * C], F16, kind="ExternalInput")
    bt_d = nc.dram_tensor("bt", [1, 128], F32, kind="ExternalInput")
    bg_d = nc.dram_tensor("bg", [1, C], F32, kind="ExternalInput")
    bf_d = nc.dram_tensor("bf", [128, 2], F32, kind="ExternalInput")
    bo1_d = nc.dram_tensor("bo1", [128, 2], F32, kind="ExternalInput")
    bo2_d = nc.dram_tensor("bo2", [128, 2], F32, kind="ExternalInput")
    out_d = nc.dram_tensor("out", [2 * C, NH], F32, kind="ExternalOutput")

    with tile.TileContext(nc) as tc:
        with (
            tc.tile_pool(name="res", bufs=1) as res,
            tc.tile_pool(name="dram", bufs=1, space="DRAM") as dram,
        ):
            # ---------------- resident tiles ----------------
            X1vT = [res.tile([128, NH], F16, name=f"x1vt{k}") for k in range(CK)]
            X2 = [res.tile([128, N], F16, name=f"x2_{k}") for k in range(CK)]
            Z1T = res.tile([128, MT * 256], BF16, name="Z1T")
            Z2T = [res.tile([128, 256], BF16, name=f"z2t{j}") for j in range(MT)]
            ones_f32 = res.tile([128, 128], F32, name="ones_f32")
            nc.vector.memset(ones_f32[:], 1.0)
            ones_bf = res.tile([128, 128], BF16, name="ones_bf")
            nc.vector.memset(ones_bf[:], 1.0)
            bf_sb = res.tile([128, 2], F32, name="bf_sb")
            bo1_sb = res.tile([128, 2], F32, name="bo1_sb")
            bo2_sb = res.tile([128, 2], F32, name="bo2_sb")
            nc.sync.dma_start(bf_sb[:], bf_d[:, :])
            nc.sync.dma_start(bo1_sb[:], bo1_d[:, :])
            nc.sync.dma_start(bo2_sb[:], bo2_d[:, :])
            neg_shift = res.tile([128, 1], F32, name="neg_shift")
            nc.vector.memset(neg_shift[:], -SHIFT)
            colsumP = res.tile([128, MT * NB], F32, name="colsumP")
            colscale = [res.tile([128, HALF], F32, name=f"colscale{h}")
                        for h in range(2)]
            btrep = res.tile([128, 128], F32, name="btrep")
            bgrep = res.tile([128, C], F32, name="bgrep")

            # warm up the CC engine: the first collective after power-on
            # pays ~11.5us of startup before ALGO_MESH_BEGIN; a dummy
            # AllReduce during phase P absorbs it off the critical path
            warm_in = dram.tile([1, 4], F32, name="warm_in")
            warm_out = dram.tile([1, 4], F32, name="warm_out")
            warm2_in = dram.tile([128, 4], BF16, name="warm2_in")
            warm2_out = dram.tile([128, 4], BF16, name="warm2_out")
            nc.gpsimd.collective_compute(
                "AllReduce", ADD,
                replica_groups=[[0, 1], [2, 3], [4, 5], [6, 7]],
                ins=[warm_in.opt()], outs=[warm_out.opt()],
            )

            # ---------------- phase P: loads + projections + Z ----------
            with tc.tile_pool(name="px", bufs=1) as px:
                bst = px.tile([1, 128], F32, name="bst")
                nc.sync.dma_start(bst[:], bt_d[:, :])
                bst2 = px.tile([1, C], F32, name="bst2")
                nc.sync.dma_start(bst2[:], bg_d[:, :])

                wtf_s = [px.tile([128, C], F16, name=f"wtf{k}") for k in range(CK)]
                wtg13_s = [px.tile([128, 384], F16, name=f"wtg13{k}")
                           for k in range(CK)]
                wo_s = [px.tile([128, 2 * C], F16, name=f"wo{k}") for k in range(CK)]
                # whole-tile DMAs spread over three engine queues: each
                # DMA trigger costs ~0.6us of sequencer time, so few big
                # transfers beat many chunked ones
                x_s = [px.tile([128, N], F16, name=f"xs{k}") for k in range(CK)]
                xp_s = [px.tile([128, N], F16, name=f"xps{k}") for k in range(CK)]
                for k in range(CK):
                    nc.scalar.dma_start(
                        wtf_s[k][:], wtf_d[128 * k:128 * (k + 1), :])
                    nc.scalar.dma_start(
                        wtg13_s[k][:], wtg13_d[128 * k:128 * (k + 1), :])
                    nc.scalar.dma_start(wo_s[k][:], wo_d[128 * k:128 * (k + 1), :])
                    nc.sync.dma_start(x_s[k][:], x_full_d[128 * k:128 * (k + 1), :])
                    nc.gpsimd.dma_start(
                        xp_s[k][:], x_mp_d[128 * k:128 * (k + 1), :])

                X3vT = [px.tile([128, N], F16, name=f"x3vt{k}") for k in range(CK)]

                with tc.tile_pool(name="pp1", bufs=1, space="PSUM") as pp1:
                    # replicated free-dim bias rows via ones-matmul
                    pbr = pp1.tile([128, C], F32, tag="pbr", bufs=1, name="pbr")
                    nc.tensor.matmul(pbr[:, 0:128], ones_f32[0:1, :], bst[0:1, :],
                                     start=True, stop=True)
                    nc.vector.tensor_copy(btrep[:], pbr[:, 0:128])
                    pbr2 = pp1.tile([128, C], F32, tag="pbr", bufs=1, name="pbr2")
                    nc.tensor.matmul(pbr2[:], ones_f32[0:1, :], bst2[0:1, :],
                                     start=True, stop=True)
                    nc.vector.tensor_copy(bgrep[:], pbr2[:])

                    # interleave the X2 and X1/X3 streams: per step one
                    # X2 tile (ACT writes) + two p13 tiles (DVE writes).
                    # X1vT/X3vT are stored t-major ([c, 128t+q]) so every
                    # projection write is CONTIGUOUS (strided SBUF writes
                    # run at ~4-8 cycles/elem on the DVE); the consumers
                    # read them through strided APs instead, which the PE
                    # streams at full rate.
                    for s in range(16):
                        j, i = s // 2, s % 2
                        p2 = pp1.tile([128, 512], F32, tag="p2", bufs=2,
                                      name=f"p2_{i}_{j}")
                        for k in range(CK):
                            nc.tensor.matmul(
                                p2[:], wtf_s[k][:, 128 * i:128 * (i + 1)],
                                xp_s[k][:, 512 * j:512 * (j + 1)],
                                start=(k == 0), stop=(k == CK - 1),
                            )
                        nc.scalar.activation(
                            X2[i][:, 512 * j:512 * (j + 1)], p2[:], IDENT,
                            bias=bf_sb[:, i:i + 1],
                        )
                        t = s
                        for ci in range(2):
                            p13 = pp1.tile([128, 384], F32, tag="p13", bufs=3,
                                           name=f"p13_{t}_{ci}")
                            for k in range(CK):
                                nc.tensor.matmul(
                                    p13[:],
                                    x_s[k][:, 256 * t + 128 * ci:
                                           256 * t + 128 * (ci + 1)],
                                    wtg13_s[k][:],
                                    start=(k == 0), stop=(k == CK - 1),
                                )
                            nc.vector.tensor_tensor(
                                X1vT[ci][:, 128 * t:128 * (t + 1)],
                                p13[:, 0:128], btrep[:], ADD)
                            nc.vector.tensor_tensor(
                                X3vT[ci][:, 256 * t:256 * (t + 1)],
                                p13[:, 128:384], bgrep[:], ADD)

                # ---------------- Z build ----------------
                # X3vT is t-major (mpos order), matching X2's column
                # order, so plain contiguous chunks line up with the
                # est m-tiles
                with tc.tile_pool(name="pzp", bufs=3, space="PSUM") as pzp:
                    for j in range(MT):
                        pzt = pzp.tile([128, 512], F32, tag="pzt", name=f"pzt{j}")
                        for k in range(CK):
                            nc.tensor.matmul(
                                pzt[:], X3vT[k][:, 128 * j:128 * (j + 1)],
                                wo_s[k][:],
                                start=(k == 0), stop=(k == CK - 1),
                            )
                        if j % 2 == 0:
                            nc.vector.tensor_copy(
                                Z1T[:, 256 * j:256 * (j + 1)], pzt[:, 0:256])
                            nc.scalar.activation(Z2T[j][:], pzt[:, 256:512], IDENT)
                        else:
                            nc.scalar.activation(
                                Z1T[:, 256 * j:256 * (j + 1)], pzt[:, 0:256],
                                IDENT)
                            nc.vector.tensor_copy(Z2T[j][:], pzt[:, 256:512])

            # ---------------- main sweep + collectives + path1 ----------
            with (
                tc.tile_pool(name="estp", bufs=1) as estp,
                tc.tile_pool(name="sw", bufs=1) as sw,
            ):
                est_t = [estp.tile([128, 512], BF16, name=f"est_{ix}")
                         for ix in range(TOT)]
                cl_t = [sw.tile([128, HALF], F32, name=f"cl{h}") for h in range(2)]
                cg_t = [sw.tile([128, HALF], F32, name=f"cg{h}") for h in range(2)]
                ar_in = [dram.tile([128, HALF], F32, name=f"ar_in{h}")
                         for h in range(2)]
                ar_out = [dram.tile([128, HALF], F32, name=f"ar_out{h}")
                          for h in range(2)]
                csview = colsumP.rearrange("p (m b) -> p m b", b=NB)

                def launch_ar(h):
                    # local colsum reduce for this half (Pool), stage to
                    # DRAM (SP queue), trigger the pairwise AllReduce
                    # (Pool; trigger only, CC engine does the work)
                    nc.vector.tensor_reduce(
                        cl_t[h][:], csview[:, HALF * h:HALF * (h + 1), :],
                        axis=mybir.AxisListType.X, op=ADD)
                    nc.sync.dma_start(ar_in[h][:], cl_t[h][:])
                    nc.gpsimd.collective_compute(
                        "AllReduce", ADD,
                        replica_groups=[[0, 1], [2, 3], [4, 5], [6, 7]],
                        ins=[ar_in[h].opt()], outs=[ar_out[h].opt()],
                    )

                with (
                    tc.tile_pool(name="pstp", bufs=3, space="PSUM") as pstp,
                    tc.tile_pool(name="paccp", bufs=2, space="PSUM") as paccp,
                    tc.tile_pool(name="prsp", bufs=1, space="PSUM") as prsp,
                ):
                    pst_tiles = {}
                    po_nb = {}
                    rowacc_nb = {}
                    pending = [None]

                    # X1vT is t-major, so contiguous 512-blocks cover a
                    # PERMUTED set of n (pos = 128t + q <-> n = 16q + t).
                    # The whole sweep runs in pos-order; the host permutes
                    # x_half on the way in and un-permutes out.
                    def issue_pst(ix):
                        nb, mj = divmod(ix, MT)
                        p = pstp.tile([128, 512], F32, tag="st", name=f"pst_{ix}")
                        for k in range(CK):
                            nc.tensor.matmul(
                                p[:], X2[k][:, 128 * mj:128 * (mj + 1)],
                                X1vT[k][:, 512 * nb:512 * (nb + 1)],
                                start=(k == 0), stop=(k == CK - 1),
                            )
                        pst_tiles[ix] = p

                    def issue_est(ix):
                        nb, mj = divmod(ix, MT)
                        col = NB * mj + nb
                        nc.scalar.activation(
                            est_t[ix][:], pst_tiles.pop(ix)[:], EXP,
                            bias=neg_shift[:],
                            accum_out=colsumP[:, col:col + 1],
                        )

                    def epilogue0(nb):
                        # deferred rowsum + path0 epilogue for block nb:
                        # prs on PE (after two fresh psts), fast-approx
                        # reciprocal on DVE, elementwise tail on Pool
                        prs = prsp.tile([128, 512], F32, tag="rs",
                                        name=f"prs{nb}")
                        nc.tensor.matmul(prs[:], ones_bf[:], rowacc_nb[nb][:],
                                         start=True, stop=True)
                        rrep = sw.tile([128, 512], F32, tag="rrep", bufs=2,
                                       name=f"rrep{nb}")
                        nc.vector.reciprocal_approx_fast(rrep[:], prs[:])
                        po = po_nb.pop(nb)
                        for i in range(2):
                            xt = sw.tile([128, 512], F32, tag="xt", bufs=2,
                                         name=f"xt0_{nb}_{i}")
                            nc.sync.dma_start(
                                xt[:], x_half_d[128 * i:128 * (i + 1),
                                                512 * nb:512 * (nb + 1)])
                            on = sw.tile([128, 512], F32, tag="on", bufs=2,
                                         name=f"on0_{nb}_{i}")
                            nc.vector.tensor_tensor(on[:], po[i][:], rrep[:], MULT)
                            nc.vector.tensor_tensor(on[:], on[:], xt[:], ADD)
                            nc.vector.tensor_scalar(
                                on[:], on[:], bo1_sb[:, i:i + 1], 0.0, ADD, MAX)
                            nc.sync.dma_start(
                                out_d[128 * i:128 * (i + 1),
                                      512 * nb:512 * (nb + 1)], on[:])

                    issue_pst(0)
                    issue_est(0)
                    issue_pst(1)
                    issue_est(1)
                    for ix in range(TOT):
                        nb, mj = divmod(ix, MT)
                        if ix + 2 < TOT:
                            issue_pst(ix + 2)
                            issue_est(ix + 2)
                        if mj == 0:
                            po_nb[nb] = [
                                paccp.tile([128, 512], F32, tag=f"po{i}", bufs=2,
                                           name=f"po{i}_{nb}") for i in range(2)]
                            rowacc_nb[nb] = sw.tile(
                                [128, 512], BF16, tag="rowacc", bufs=2,
                                name=f"rowacc{nb}")
                            nc.vector.tensor_copy(rowacc_nb[nb][:], est_t[ix][:])
                        else:
                            nc.vector.tensor_tensor(
                                rowacc_nb[nb][:], rowacc_nb[nb][:],
                                est_t[ix][:], ADD)
                        if mj == 1 and pending[0] is not None:
                            pending[0]()
                            pending[0] = None
                        if ix == TOT - (MT - HALF) + 1:
                            launch_ar(0)
                        if ix == 56:
                            # keep the CC engine warm ahead of the real
                            # AllReduces; gated on sweep data so it
                            # actually fires mid-sweep
                            wsb = sw.tile([128, 4], BF16, name="warm_sb")
                            nc.gpsimd.tensor_copy(wsb[:], est_t[54][:, 0:4])
                            nc.gpsimd.dma_start(warm2_in[:], wsb[:])
                            nc.gpsimd.collective_compute(
                                "AllReduce", ADD,
                                replica_groups=[[0, 1], [2, 3], [4, 5], [6, 7]],
                                ins=[warm2_in.opt()], outs=[warm2_out.opt()],
                            )
                        for i in range(2):
                            nc.tensor.matmul(
                                po_nb[nb][i][:],
                                Z1T[:, 256 * mj + 128 * i:
                                    256 * mj + 128 * (i + 1)],
                                est_t[ix][:],
                                start=(mj == 0), stop=(mj == MT - 1),
                            )
                        if mj == MT - 1:
                            pending[0] = (lambda nb=nb: epilogue0(nb))

                    # second-half AllReduce, then the deferred last
                    # epilogue overlaps its latency
                    launch_ar(1)
                    # cg DMA-backs on the Pool queue: AR1 is done (or
                    # nearly) by now, so the wait doesn't block the
                    # Z2 scales queued behind it
                    nc.gpsimd.dma_start(cg_t[0][:], ar_out[0][:])
                    nc.vector.reciprocal_approx_fast(colscale[0][:], cg_t[0][:])
                    for j in range(HALF):
                        nc.vector.tensor_scalar_mul(
                            Z2T[j][:], Z2T[j][:], colscale[0][:, j:j + 1])
                    pending[0]()
                    pending[0] = None
                    nc.gpsimd.dma_start(cg_t[1][:], ar_out[1][:])
                    nc.vector.reciprocal_approx_fast(colscale[1][:], cg_t[1][:])
                    for j in range(HALF, MT):
                        nc.vector.tensor_scalar_mul(
                            Z2T[j][:], Z2T[j][:],
                            colscale[1][:, j - HALF:j - HALF + 1])

                # ---------------- path 1: pure matmul sweep --------------
                # all four n-blocks' accumulations stay open across the 8
                # PSUM banks; mj 0..15 runs while AR2 is still in flight
                with tc.tile_pool(name="pacc2", bufs=1, space="PSUM") as pacc2:
                    po1 = [[pacc2.tile([128, 512], F32, tag=f"q{nb}_{i}", bufs=1,
                                       name=f"q{nb}_{i}") for i in range(2)]
                           for nb in range(NB)]
                    # first half mj-outer across all nb (hides AR2 in
                    # flight), second half nb-major so each nb's epilogue
                    # overlaps the next nb's matmuls
                    for mj in range(HALF):
                        for i in range(2):
                            for nb in range(NB):
                                nc.tensor.matmul(
                                    po1[nb][i][:],
                                    Z2T[mj][:, 128 * i:128 * (i + 1)],
                                    est_t[MT * nb + mj][:],
                                    start=(mj == 0), stop=False,
                                )
                    for nb in range(NB):
                        for mj in range(HALF, MT):
                            for i in range(2):
                                nc.tensor.matmul(
                                    po1[nb][i][:],
                                    Z2T[mj][:, 128 * i:128 * (i + 1)],
                                    est_t[MT * nb + mj][:],
                                    start=False, stop=(mj == MT - 1),
                                )
                        for i in range(2):
                            xt = sw.tile([128, 512], F32, tag="xt", bufs=2,
                                         name=f"xt1_{nb}_{i}")
                            nc.sync.dma_start(
                                xt[:], x_half_d[128 * i:128 * (i + 1),
                                                512 * nb:512 * (nb + 1)])
                            on = sw.tile([128, 512], F32, tag="on", bufs=2,
                                         name=f"on1_{nb}_{i}")
                            nc.vector.tensor_tensor(
                                on[:], po1[nb][i][:], xt[:], ADD)
                            nc.vector.tensor_scalar(
                                on[:], on[:], bo2_sb[:, i:i + 1], 0.0, ADD, MAX)
                            nc.sync.dma_start(
                                out_d[C + 128 * i:C + 128 * (i + 1),
                                      512 * nb:512 * (nb + 1)], on[:])

    nc.compile()
    return nc


# n-axis permutation of the device's local pixel columns: device column
# pos holds natural local pixel n = 16*(pos%128) + pos//128 (t-major
# X1vT storage). x_half is permuted on input, out un-permuted on gather.
_PERM = 16 * (np.arange(NH) % 128) + np.arange(NH) // 128
# m-axis (full pixel range) t-major permutation for the X2 input copy
_PERM_M = 16 * (np.arange(N) % 256) + np.arange(N) // 256


def _in_maps(x, w_teta, b_teta, w_fi, b_fi, w_gi, b_gi, w_o1, b_o1, w_o2, b_o2):
    xf = np.ascontiguousarray(x.reshape(B, C, N), dtype=np.float32)
    xf16 = xf.astype(np.float16)
    wtf = np.ascontiguousarray(w_fi.T, dtype=np.float16)
    wtgT = np.asarray(w_gi.T, dtype=np.float32)
    wo = np.ascontiguousarray(
        np.concatenate([w_o1.T, w_o2.T], axis=1), dtype=np.float16)
    bf = np.ascontiguousarray(b_fi.reshape(2, 128).T, dtype=np.float32)
    bo1 = np.ascontiguousarray(b_o1.reshape(2, 128).T, dtype=np.float32)
    bo2 = np.ascontiguousarray(b_o2.reshape(2, 128).T, dtype=np.float32)
    bg = np.ascontiguousarray(b_gi.reshape(1, C), dtype=np.float32)
    wtetaT = np.asarray(w_teta.T, dtype=np.float32)
    maps = []
    for c in range(8):
        b, r = c // 2, c % 2
        # rotate so the local q-half sits in columns 0:128, keep only it
        wtt_loc = np.roll(wtetaT, -128 * r, axis=1)[:, 0:128]
        wtg13 = np.ascontiguousarray(
            np.concatenate([wtt_loc, wtgT], axis=1), dtype=np.float16)
        maps.append({
            "x_full": xf16[b],
            "x_mp": np.ascontiguousarray(xf16[b][:, _PERM_M]),
            "x_half": np.ascontiguousarray(
                xf[b][:, NH * r:NH * (r + 1)][:, _PERM]),
            "wtg13": wtg13,
            "wtf": wtf, "wo": wo,
            "bt": np.ascontiguousarray(
                b_teta[128 * r:128 * (r + 1)].reshape(1, 128), dtype=np.float32),
            "bg": bg, "bf": bf, "bo1": bo1, "bo2": bo2,
        })
    return maps


def run(trace=False, **inputs):
    if "nc" not in _CACHE:
        _CACHE["nc"] = _build_nc()
    nc = _CACHE["nc"]
    maps = _in_maps(**inputs)
    res = run_bass_kernel_spmd(nc, maps, core_ids=list(range(8)), trace=trace)
    out = np.empty((B, 2 * C, N), dtype=np.float32)
    for c in range(8):
        b, r = c // 2, c % 2
        blk = out[b][:, NH * r:NH * (r + 1)]
        blk[:, _PERM] = res.results[c]["out"]
    return out.reshape(B, 2 * C, H, W), res


def kernel(**inputs):
    out, _ = run(trace=False, **inputs)
    return out
